# revision 6
# baseline (speedup 1.0000x reference)
"""Trainium2 Bass kernel for nn_AttnDecoder (attention decoder step).

Reference computation (B=64, S=512, H=1024, E=512, V=32000):
  emb     = emb_table[x]                                  [B, E]
  energy  = relu(cat(h_rep, enc) @ energy_W.T + be)       [S, B]
  attn    = softmax(energy, axis=S)
  context = sum_s attn * enc                              [B, 2H]
  gates   = [context, emb] @ W_ih.T + h @ W_hh.T + b      [B, 4H]
  i,f,o   = sigmoid(...), g = tanh(...)
  c'      = f*c + i*g ;  h' = o*tanh(c')
  preds   = h' @ fc_W.T + fc_b                            [B, V]

Sharding over 8 cores:
  - attention: data-parallel over batch (8 rows per core)
  - AllGather(context)
  - LSTM: tensor-parallel over gate dim (128 rows per gate per core),
    computed in [j, b] layout so the h' slices AllGather directly into
    the transposed [H, B] layout the fc matmul needs as stationary
  - fc: tensor-parallel over vocab (4000 cols per core)

All matmul inputs bf16 (fp32 accumulation in PSUM); softmax/LSTM state fp32.
"""
import numpy as np
import ml_dtypes

import concourse.bacc as bacc
import concourse.tile as tile
import concourse.mybir as mybir
from concourse.bass_utils import run_bass_kernel_spmd

BF16 = ml_dtypes.bfloat16
N_CORES = 8
S, B, H, E, V = 512, 64, 1024, 512, 32000
D2 = 2 * H               # 2048, encoder feature dim
B_LOC = B // N_CORES     # 8 batch rows per core
GJ = H // N_CORES        # 128 rows per gate per core
V_LOC = V // N_CORES     # 4000 vocab cols per core
R = D2 + E + H           # 3584 rnn_in dim (ctx + emb + hid)
RC = R // 128            # 28 contraction chunks
SC = S // 128            # 4 seq chunks

_CACHE = {}


def _build():
    dt = mybir.dt
    nc = bacc.Bacc("TRN2", target_bir_lowering=False, debug=False,
                   num_devices=N_CORES)

    # ---- per-core external inputs
    enc_d = nc.dram_tensor("enc", [B_LOC, SC, 128, D2], dt.bfloat16, kind="ExternalInput")
    w2b_d = nc.dram_tensor("w2b", [128, D2], dt.bfloat16, kind="ExternalInput")
    w1p_d = nc.dram_tensor("w1p", [128, 8], dt.bfloat16, kind="ExternalInput")
    be_d = nc.dram_tensor("be", [1, B_LOC], dt.bfloat16, kind="ExternalInput")
    embT_d = nc.dram_tensor("embT", [E // 128, 128, B], dt.bfloat16, kind="ExternalInput")
    hidT_d = nc.dram_tensor("hidT", [H // 128, 128, B], dt.bfloat16, kind="ExternalInput")
    hq_d = nc.dram_tensor("hq", [H // 128, 128, B_LOC], dt.bfloat16, kind="ExternalInput")
    cellT_d = nc.dram_tensor("cellT", [128, B], dt.float32, kind="ExternalInput")
    wt_d = nc.dram_tensor("wt", [RC, 128, 512], dt.bfloat16, kind="ExternalInput")
    biasg_d = nc.dram_tensor("biasg", [1, 512], dt.bfloat16, kind="ExternalInput")
    fcw_d = nc.dram_tensor("fcw", [H // 128, 128, V_LOC], dt.bfloat16, kind="ExternalInput")
    fcb_d = nc.dram_tensor("fcb", [1, V_LOC], dt.bfloat16, kind="ExternalInput")
    i64_d = nc.dram_tensor("i64", [64, 64], dt.bfloat16, kind="ExternalInput")

    # ---- per-core external outputs
    preds_d = nc.dram_tensor("preds", [B, V_LOC], dt.float32, kind="ExternalOutput")
    hT_d = nc.dram_tensor("hT", [128, B], dt.float32, kind="ExternalOutput")
    cT_d = nc.dram_tensor("cT", [128, B], dt.float32, kind="ExternalOutput")

    with tile.TileContext(nc) as tc:
        with (
            tc.tile_pool(name="const", bufs=1) as cpool,
            tc.tile_pool(name="enc", bufs=12) as epool,
            tc.tile_pool(name="work", bufs=3) as wpool,
            tc.tile_pool(name="dump", bufs=2) as dpool,
            tc.tile_pool(name="big", bufs=1) as bpool,
            tc.tile_pool(name="out", bufs=3) as opool,
            tc.tile_pool(name="dram", bufs=1, space="DRAM") as drpool,
        ):
            f32, bf = dt.float32, dt.bfloat16

            # ======== constants / prefetches ========
            w2b = cpool.tile([128, D2], bf)
            nc.sync.dma_start(w2b[:], w2b_d[:])
            w1p = cpool.tile([128, 8], bf)
            nc.sync.dma_start(w1p[:], w1p_d[:])
            be = cpool.tile([1, B_LOC], bf)
            nc.sync.dma_start(be[:], be_d[:])
            hq = cpool.tile([128, H // 128, B_LOC], bf)
            for i in range(H // 128):
                nc.sync.dma_start(hq[:, i, :], hq_d[i])
            i64 = cpool.tile([64, 64], bf)
            nc.sync.dma_start(i64[:], i64_d[:])
            biasg = cpool.tile([1, 512], bf)
            nc.sync.dma_start(biasg[:], biasg_d[:])
            fcb = cpool.tile([1, V_LOC], bf)
            nc.sync.dma_start(fcb[:], fcb_d[:])

            ones_c = cpool.tile([128, 1], bf)
            nc.vector.memset(ones_c[:], 1.0)
            ones_r = cpool.tile([1, 128], bf)
            nc.vector.memset(ones_r[:], 1.0)

            # rnn_inT rows: [0:16) ctx (filled later), [16:20) emb, [20:28) hid
            rT = bpool.tile([128, RC, B], bf)
            for i in range(E // 128):
                nc.sync.dma_start(rT[:, 16 + i, :], embT_d[i])
            for i in range(H // 128):
                nc.sync.dma_start(rT[:, 20 + i, :], hidT_d[i])

            cellT = cpool.tile([128, B], f32)
            nc.sync.dma_start(cellT[:], cellT_d[:])

            # LSTM weights + fc weights: tiles declared here, DMAs interleaved
            # into the attention loop so enc loads aren't queued behind them.
            wt = bpool.tile([128, RC, 512], bf)
            fcw = bpool.tile([128, H // 128, V_LOC], bf)
            prefetch = ([("wt", rc) for rc in range(RC)]
                        + [("fcw", c) for c in range(H // 128)])

            # ======== attention phase (own PSUM scope) ========
            pp_cm = tc.tile_pool(name="psA", bufs=1, space="PSUM")
            pp = pp_cm.__enter__()
            # ---- q[b] = hid_local . w1 + be, broadcast to [128, B_LOC]
            q_ps = pp.tile([1, B_LOC], f32, tag="q")
            nc.tensor.matmul(q_ps[:], ones_r[:, :1], be[:], start=True, stop=False)
            for c in range(H // 128):
                nc.tensor.matmul(q_ps[:], w1p[:, c:c + 1], hq[:, c, :],
                                 start=False, stop=(c == H // 128 - 1))
            q_sb = cpool.tile([1, B_LOC], bf)
            nc.vector.tensor_copy(q_sb[:], q_ps[:])
            qb_ps = pp.tile([128, B_LOC], f32, tag="qb")
            nc.tensor.matmul(qb_ps[:], ones_r[:], q_sb[:], start=True, stop=True)
            q_bc = cpool.tile([128, B_LOC], f32)
            nc.vector.tensor_copy(q_bc[:], qb_ps[:])

            # ======== attention (batch rows 8k..8k+8 of this core) ========
            # A holds exp-weights: [128 s, (b, sc), 8] masked columns
            A = cpool.tile([128, B_LOC * SC, B_LOC], bf)
            nc.vector.memset(A[:], 0.0)

            ctx_ps = pp.tile([B_LOC, D2], f32, tag="ctx")
            z_ps = pp.tile([B_LOC, 1], f32, tag="z")

            npf = (len(prefetch) + B_LOC - 1) // B_LOC
            for b in range(B_LOC):
                enc_t = []
                for sc in range(SC):
                    t = epool.tile([128, D2], bf, tag="enc")
                    nc.sync.dma_start(t[:], enc_d[b, sc])
                    enc_t.append(t)
                if b >= 1:   # keep head of queue clear for the first batch rows
                    for kind, j in prefetch[(b - 1) * npf: b * npf]:
                        if kind == "wt":
                            nc.sync.dma_start(wt[:, j, :], wt_d[j])
                        else:
                            nc.sync.dma_start(fcw[:, j, :], fcw_d[j])

                # energy: e[s] = sum_d enc[s, d] * w2[d]   (fp32 accum)
                e_raw = wpool.tile([128, SC], f32, tag="eraw")
                for sc in range(SC):
                    prod = wpool.tile([128, D2], bf, tag="prod")
                    nc.vector.tensor_mul(prod[:], enc_t[sc][:], w2b[:])
                    if sc == SC - 1:
                        nc.vector.tensor_reduce(
                            e_raw[:, sc:sc + 1], prod[:],
                            axis=mybir.AxisListType.X, op=mybir.AluOpType.add)
                    else:
                        dump = dpool.tile([128, D2], bf, tag="dump")
                        nc.scalar.activation(
                            dump[:], prod[:], mybir.ActivationFunctionType.Copy,
                            accum_out=e_raw[:, sc:sc + 1])

                # p = exp(e + q_b); softmax weight = max(p, 1) == exp(relu(e + q_b))
                p_exp = wpool.tile([128, SC], f32, tag="pexp")
                nc.scalar.activation(p_exp[:], e_raw[:],
                                     mybir.ActivationFunctionType.Exp,
                                     bias=q_bc[:, b:b + 1])
                for sc in range(SC):
                    nc.vector.tensor_scalar_max(
                        A[:, b * SC + sc, b:b + 1], p_exp[:, sc:sc + 1], 1.0)

                # context (+ z column): PSUM-accumulated masked matmuls
                first = (b == 0)
                last = (b == B_LOC - 1)
                for sc in range(SC):
                    for nn in range(D2 // 512):
                        nc.tensor.matmul(
                            ctx_ps[:, nn * 512:(nn + 1) * 512],
                            A[:, b * SC + sc, :],
                            enc_t[sc][:, nn * 512:(nn + 1) * 512],
                            start=(first and sc == 0), stop=(last and sc == SC - 1))
                    nc.tensor.matmul(
                        z_ps[:], A[:, b * SC + sc, :], ones_c[:],
                        start=(first and sc == 0), stop=(last and sc == SC - 1))

            for kind, j in prefetch[(B_LOC - 1) * npf:]:
                if kind == "wt":
                    nc.sync.dma_start(wt[:, j, :], wt_d[j])
                else:
                    nc.sync.dma_start(fcw[:, j, :], fcw_d[j])

            # normalize context, cast bf16
            inv_z = cpool.tile([B_LOC, 1], f32)
            nc.vector.reciprocal(inv_z[:], z_ps[:])
            ctx_bf = cpool.tile([B_LOC, D2], bf)
            nc.vector.tensor_scalar_mul(ctx_bf[:], ctx_ps[:], inv_z[:])

            pp_cm.__exit__(None, None, None)

            # PE warm-keeper during the ctx AllGather (HAM stays at 2.4GHz)
            warm_cm = tc.tile_pool(name="psW", bufs=1, space="PSUM")
            warmp = warm_cm.__enter__()
            warm_ps = warmp.tile([1, 128], f32, tag="warm")
            for w in range(80):
                nc.tensor.matmul(warm_ps[:], ones_r[:, :1], ones_r[:],
                                 start=True, stop=True)
            warm_cm.__exit__(None, None, None)

            # ======== AllGather context -> [B, D2] ========
            agc_in = drpool.tile([B_LOC, D2], bf)
            agc_out = drpool.tile([B, D2], bf)
            nc.sync.dma_start(agc_in[:], ctx_bf[:])
            nc.gpsimd.collective_compute(
                "AllGather", mybir.AluOpType.bypass,
                replica_groups=[list(range(N_CORES))],
                ins=[agc_in[:]], outs=[agc_out[:]])
            ctx_full = cpool.tile([B, D2], bf)
            nc.sync.dma_start(ctx_full[:], agc_out[:])

            # ======== LSTM phase (own PSUM scope) ========
            ppg_cm = tc.tile_pool(name="psB", bufs=1, space="PSUM")
            ppg = ppg_cm.__enter__()
            # transpose ctx into rnn_inT rows [0:16)
            for rc in range(D2 // 128):
                tp_ps = ppg.tile([128, B], bf, tag=f"tp{rc % 2}", name=f"tp_ps{rc}")
                nc.tensor.transpose(tp_ps[:], ctx_full[:, rc * 128:(rc + 1) * 128], i64[:])
                nc.vector.tensor_copy(rT[:, rc, :], tp_ps[:])

            # ======== gates: [j, b] = sum_r Wcat[j, r] * rnn_in[r, b] ========
            g_ps = [ppg.tile([128, B], f32, tag=f"g{jc}", name=f"g_ps{jc}")
                    for jc in range(4)]
            for jc in range(4):
                nc.tensor.matmul(g_ps[jc][:], biasg[:, jc * 128:(jc + 1) * 128],
                                 ones_r[:, :B], start=True, stop=False)
                for rc in range(RC):
                    nc.tensor.matmul(g_ps[jc][:],
                                     wt[:, rc, jc * 128:(jc + 1) * 128],
                                     rT[:, rc, :],
                                     start=False, stop=(rc == RC - 1))

            # ======== LSTM cell (j-slice on partitions, b on free) ========
            Tanh = mybir.ActivationFunctionType.Tanh
            sig = []
            for idx, jc in enumerate((0, 1, 3)):       # i, f, o gates
                t = opool.tile([128, B], f32, tag=f"sg{idx}")
                nc.scalar.activation(t[:], g_ps[jc][:], Tanh, scale=0.5)
                nc.vector.tensor_scalar(t[:], t[:], 0.5, 0.5,
                                        op0=mybir.AluOpType.mult,
                                        op1=mybir.AluOpType.add)
                sig.append(t)
            i_s, f_s, o_s = sig
            g_t = opool.tile([128, B], f32, tag="gt")
            nc.scalar.activation(g_t[:], g_ps[2][:], Tanh)

            fc_ = opool.tile([128, B], f32, tag="fc_")
            nc.vector.tensor_mul(fc_[:], f_s[:], cellT[:])
            ig = opool.tile([128, B], f32, tag="ig")
            nc.vector.tensor_mul(ig[:], i_s[:], g_t[:])
            c_new = opool.tile([128, B], f32, tag="cnew")
            nc.vector.tensor_add(c_new[:], fc_[:], ig[:])
            nc.sync.dma_start(cT_d[:], c_new[:])

            tc_t = opool.tile([128, B], f32, tag="tct")
            nc.scalar.activation(tc_t[:], c_new[:], Tanh)
            h_new = opool.tile([128, B], f32, tag="hnew")
            nc.vector.tensor_mul(h_new[:], o_s[:], tc_t[:])
            nc.sync.dma_start(hT_d[:], h_new[:])
            h_bf = opool.tile([128, B], bf, tag="hbf")
            nc.vector.tensor_copy(h_bf[:], h_new[:])

            # ======== AllGather h'^T -> [H, B] ========
            agh_in = drpool.tile([128, B], bf)
            agh_out = drpool.tile([H, B], bf)
            nc.sync.dma_start(agh_in[:], h_bf[:])
            nc.gpsimd.collective_compute(
                "AllGather", mybir.AluOpType.bypass,
                replica_groups=[list(range(N_CORES))],
                ins=[agh_in[:]], outs=[agh_out[:]])
            warm2_cm = tc.tile_pool(name="psW2", bufs=1, space="PSUM")
            warmp2 = warm2_cm.__enter__()
            warm2_ps = warmp2.tile([1, 128], f32, tag="warm2")
            for w in range(60):
                nc.tensor.matmul(warm2_ps[:], ones_r[:, :1], ones_r[:],
                                 start=True, stop=True)
            warm2_cm.__exit__(None, None, None)

            hT_full = cpool.tile([128, H // 128, B], bf)
            for c in range(H // 128):
                nc.sync.dma_start(hT_full[:, c, :],
                                  agh_out[c * 128:(c + 1) * 128, :])

            ppg_cm.__exit__(None, None, None)

            # ======== fc phase (own PSUM scope) ========
            ppfc_cm = tc.tile_pool(name="psC", bufs=2, space="PSUM")
            ppfc = ppfc_cm.__enter__()
            NV = V_LOC // 500
            for nn in range(NV):
                fc_ps = ppfc.tile([B, 500], f32, tag="fcps")
                nc.tensor.matmul(fc_ps[:], ones_r[:, :B],
                                 fcb[:, nn * 500:(nn + 1) * 500],
                                 start=True, stop=False)
                for c in range(H // 128):
                    nc.tensor.matmul(fc_ps[:], hT_full[:, c, :],
                                     fcw[:, c, nn * 500:(nn + 1) * 500],
                                     start=False, stop=(c == H // 128 - 1))
                p_sb = opool.tile([B, 500], f32, tag="psb")
                nc.scalar.copy(p_sb[:], fc_ps[:])
                nc.sync.dma_start(preds_d[:, nn * 500:(nn + 1) * 500], p_sb[:])
            ppfc_cm.__exit__(None, None, None)

    nc.compile()
    return nc


def _prep_inputs(x, encoder_states, hidden, cell, emb_table, energy_W, energy_b,
                 W_ih, W_hh, b_ih, b_hh, fc_W, fc_b):
    """Shard + lay out the full inputs for the 8 cores (host-side, layout only)."""
    enc = np.asarray(encoder_states, np.float32)          # [S, B, D2]
    hid = np.asarray(hidden, np.float32)[0]               # [B, H]
    cel = np.asarray(cell, np.float32)[0]                 # [B, H]
    eW = np.asarray(energy_W, np.float32)[0]              # [3H]
    emb = np.asarray(emb_table, np.float32)[np.asarray(x).astype(np.int64)]  # [B, E]

    w1, w2 = eW[:H], eW[H:]
    w1p = np.ascontiguousarray(w1.reshape(8, 128).T).astype(BF16)        # [128, 8]
    w2b = np.broadcast_to(w2.astype(BF16), (128, D2))                    # [128, D2]
    w2b = np.ascontiguousarray(w2b)
    be = np.full((1, B_LOC), float(np.asarray(energy_b).reshape(-1)[0]), BF16)

    embT = np.ascontiguousarray(emb.T).astype(BF16).reshape(E // 128, 128, B)
    hidT = np.ascontiguousarray(hid.T).astype(BF16).reshape(H // 128, 128, B)

    Wcat = np.concatenate([np.asarray(W_ih, np.float32),
                           np.asarray(W_hh, np.float32)], axis=1)        # [4H, R]
    bias = np.asarray(b_ih, np.float32) + np.asarray(b_hh, np.float32)   # [4H]
    fcW = np.asarray(fc_W, np.float32)                                   # [V, H]
    fcb = np.asarray(fc_b, np.float32)                                   # [V]
    i64 = np.eye(64, dtype=BF16)

    enc_bf = enc.astype(BF16)                                            # [S, B, D2]

    in_maps = []
    for k in range(N_CORES):
        bsl = slice(k * B_LOC, (k + 1) * B_LOC)
        enc_k = np.ascontiguousarray(
            enc_bf[:, bsl, :].transpose(1, 0, 2)).reshape(B_LOC, SC, 128, D2)
        jidx = np.concatenate([np.arange(g * H + k * GJ, g * H + (k + 1) * GJ)
                               for g in range(4)])                        # [512]
        wt_k = np.ascontiguousarray(Wcat[jidx].T).astype(BF16).reshape(RC, 128, 512)
        biasg_k = bias[jidx].astype(BF16).reshape(1, 512)
        cellT_k = np.ascontiguousarray(cel[:, k * GJ:(k + 1) * GJ].T).astype(np.float32)
        vsl = slice(k * V_LOC, (k + 1) * V_LOC)
        fcw_k = np.ascontiguousarray(fcW[vsl].T).astype(BF16).reshape(H // 128, 128, V_LOC)
        fcb_k = fcb[vsl].astype(BF16).reshape(1, V_LOC)
        hq_k = np.ascontiguousarray(hidT[:, :, bsl])
        in_maps.append({
            "enc": enc_k, "w2b": w2b, "w1p": w1p, "be": be, "hq": hq_k,
            "embT": embT, "hidT": hidT, "cellT": cellT_k,
            "wt": wt_k, "biasg": biasg_k,
            "fcw": fcw_k, "fcb": fcb_k, "i64": i64,
        })
    return in_maps


def kernel(**inputs):
    if "nc" not in _CACHE:
        _CACHE["nc"] = _build()
    nc = _CACHE["nc"]
    in_maps = _prep_inputs(**inputs)
    res = run_bass_kernel_spmd(nc, in_maps, core_ids=list(range(N_CORES)),
                               trace=False)
    preds = np.concatenate([res.results[k]["preds"] for k in range(N_CORES)],
                           axis=1)                                       # [B, V]
    h_new = np.concatenate([res.results[k]["hT"].T for k in range(N_CORES)],
                           axis=1)[None]                                 # [1, B, H]
    c_new = np.concatenate([res.results[k]["cT"].T for k in range(N_CORES)],
                           axis=1)[None]
    return preds, h_new, c_new


# revision 8
# speedup vs baseline: 1.0270x; 1.0270x over previous
"""Trainium2 Bass kernel for nn_AttnDecoder (attention decoder step).

Reference computation (B=64, S=512, H=1024, E=512, V=32000):
  emb     = emb_table[x]                                  [B, E]
  energy  = relu(cat(h_rep, enc) @ energy_W.T + be)       [S, B]
  attn    = softmax(energy, axis=S)
  context = sum_s attn * enc                              [B, 2H]
  gates   = [context, emb] @ W_ih.T + h @ W_hh.T + b      [B, 4H]
  i,f,o   = sigmoid(...), g = tanh(...)
  c'      = f*c + i*g ;  h' = o*tanh(c')
  preds   = h' @ fc_W.T + fc_b                            [B, V]

Sharding over 8 cores:
  - attention: data-parallel over batch (8 rows per core); context is
    AllGathered in two half-batch rounds so the first gather overlaps the
    second half of the attention compute
  - LSTM: tensor-parallel over gate dim (128 rows per gate per core),
    computed in [j, b] layout so the h' slices AllGather directly into
    the transposed [H, B] layout the fc matmul needs as stationary
  - fc: tensor-parallel over vocab (4000 cols per core)

All matmul inputs bf16 (fp32 accumulation in PSUM); softmax/LSTM state fp32.
"""
import numpy as np
import ml_dtypes

import concourse.bacc as bacc
import concourse.tile as tile
import concourse.mybir as mybir
from concourse.bass_utils import run_bass_kernel_spmd

BF16 = ml_dtypes.bfloat16
N_CORES = 8
S, B, H, E, V = 512, 64, 1024, 512, 32000
D2 = 2 * H               # 2048, encoder feature dim
B_LOC = B // N_CORES     # 8 batch rows per core
HB = B_LOC // 2          # 4, half-batch AllGather granule
GJ = H // N_CORES        # 128 rows per gate per core
V_LOC = V // N_CORES     # 4000 vocab cols per core
R = D2 + E + H           # 3584 rnn_in dim (ctx + emb + hid)
RC = R // 128            # 28 contraction chunks
SC = S // 128            # 4 seq chunks

_CACHE = {}


def _build():
    dt = mybir.dt
    nc = bacc.Bacc("TRN2", target_bir_lowering=False, debug=False,
                   num_devices=N_CORES)

    # ---- per-core external inputs
    enc_d = nc.dram_tensor("enc", [B_LOC, SC, 128, D2], dt.bfloat16, kind="ExternalInput")
    w2b_d = nc.dram_tensor("w2b", [128, D2], dt.bfloat16, kind="ExternalInput")
    w1p_d = nc.dram_tensor("w1p", [128, 8], dt.bfloat16, kind="ExternalInput")
    be_d = nc.dram_tensor("be", [1, B_LOC], dt.bfloat16, kind="ExternalInput")
    embT_d = nc.dram_tensor("embT", [E // 128, 128, B], dt.bfloat16, kind="ExternalInput")
    hidT_d = nc.dram_tensor("hidT", [H // 128, 128, B], dt.bfloat16, kind="ExternalInput")
    hq_d = nc.dram_tensor("hq", [H // 128, 128, B_LOC], dt.bfloat16, kind="ExternalInput")
    cellT_d = nc.dram_tensor("cellT", [128, B], dt.float32, kind="ExternalInput")
    wt_d = nc.dram_tensor("wt", [RC, 128, 512], dt.bfloat16, kind="ExternalInput")
    biasg_d = nc.dram_tensor("biasg", [1, 512], dt.bfloat16, kind="ExternalInput")
    fcw_d = nc.dram_tensor("fcw", [H // 128, 128, V_LOC], dt.bfloat16, kind="ExternalInput")
    fcb_d = nc.dram_tensor("fcb", [1, V_LOC], dt.bfloat16, kind="ExternalInput")
    i64_d = nc.dram_tensor("i64", [64, 64], dt.bfloat16, kind="ExternalInput")

    # ---- per-core external outputs
    preds_d = nc.dram_tensor("preds", [B, V_LOC], dt.float32, kind="ExternalOutput")
    hT_d = nc.dram_tensor("hT", [128, B], dt.float32, kind="ExternalOutput")
    cT_d = nc.dram_tensor("cT", [128, B], dt.float32, kind="ExternalOutput")

    f32, bf = dt.float32, dt.bfloat16
    Exp = mybir.ActivationFunctionType.Exp
    Copy = mybir.ActivationFunctionType.Copy
    Tanh = mybir.ActivationFunctionType.Tanh

    with tile.TileContext(nc) as tc:
        with (
            tc.tile_pool(name="const", bufs=1) as cpool,
            tc.tile_pool(name="enc", bufs=10) as epool,
            tc.tile_pool(name="work", bufs=3) as wpool,
            tc.tile_pool(name="dump", bufs=1) as dpool,
            tc.tile_pool(name="big", bufs=1) as bpool,
            tc.tile_pool(name="out", bufs=3) as opool,
            tc.tile_pool(name="dram", bufs=1, space="DRAM") as drpool,
        ):
            # ======== constants / small loads ========
            w2b = cpool.tile([128, D2], bf)
            nc.sync.dma_start(w2b[:], w2b_d[:])
            w1p = cpool.tile([128, 8], bf)
            nc.sync.dma_start(w1p[:], w1p_d[:])
            be = cpool.tile([1, B_LOC], bf)
            nc.sync.dma_start(be[:], be_d[:])
            hq = cpool.tile([128, H // 128, B_LOC], bf)
            for i in range(H // 128):
                nc.sync.dma_start(hq[:, i, :], hq_d[i])
            i64 = cpool.tile([64, 64], bf)
            nc.sync.dma_start(i64[:], i64_d[:])
            biasg = cpool.tile([1, 512], bf)
            nc.sync.dma_start(biasg[:], biasg_d[:])
            fcb = cpool.tile([1, V_LOC], bf)
            nc.sync.dma_start(fcb[:], fcb_d[:])
            cellT = cpool.tile([128, B], f32)
            nc.sync.dma_start(cellT[:], cellT_d[:])

            ones_c = cpool.tile([128, 1], bf)
            nc.vector.memset(ones_c[:], 1.0)
            ones_r = cpool.tile([1, 128], bf)
            nc.vector.memset(ones_r[:], 1.0)

            # rnn_inT rows: [0:16) ctx (filled post-gather), [16:20) emb, [20:28) hid
            rT = bpool.tile([128, RC, B], bf)
            for i in range(E // 128):
                nc.sync.dma_start(rT[:, 16 + i, :], embT_d[i])
            for i in range(H // 128):
                nc.sync.dma_start(rT[:, 20 + i, :], hidT_d[i])

            # LSTM + fc weights: DMAs interleaved into the attention loop so
            # the first enc tiles aren't queued behind 12MB of prefetch.
            wt = bpool.tile([128, RC, 512], bf)
            fcw = bpool.tile([128, H // 128, V_LOC], bf)
            prefetch = ([("wt", rc) for rc in range(RC)]
                        + [("fcw", c) for c in range(H // 128)])

            def emit_prefetch(lo, hi):
                for kind, j in prefetch[lo:hi]:
                    if kind == "wt":
                        nc.sync.dma_start(wt[:, j, :], wt_d[j])
                    else:
                        nc.sync.dma_start(fcw[:, j, :], fcw_d[j])

            # ======== q[b] = hid_local . w1 + be  (tiny PSUM scope) ========
            ppq_cm = tc.tile_pool(name="psQ", bufs=1, space="PSUM")
            ppq = ppq_cm.__enter__()
            q_ps = ppq.tile([1, B_LOC], f32, tag="q")
            nc.tensor.matmul(q_ps[:], ones_r[:, :1], be[:], start=True, stop=False)
            for c in range(H // 128):
                nc.tensor.matmul(q_ps[:], w1p[:, c:c + 1], hq[:, c, :],
                                 start=False, stop=(c == H // 128 - 1))
            q_sb = cpool.tile([1, B_LOC], bf)
            nc.vector.tensor_copy(q_sb[:], q_ps[:])
            qb_ps = ppq.tile([128, B_LOC], f32, tag="qb")
            nc.tensor.matmul(qb_ps[:], ones_r[:], q_sb[:], start=True, stop=True)
            q_bc = cpool.tile([128, B_LOC], f32)
            nc.vector.tensor_copy(q_bc[:], qb_ps[:])
            ppq_cm.__exit__(None, None, None)

            # ======== attention ========
            # A holds exp-weights: [128 s, (b, sc)] masked columns (b mod HB)
            A = cpool.tile([128, B_LOC * SC, HB], bf)
            nc.vector.memset(A[:], 0.0)

            ppa_cm = tc.tile_pool(name="psA", bufs=1, space="PSUM")
            ppa = ppa_cm.__enter__()
            ctx_ps = ppa.tile([HB, D2], f32, tag="ctx")    # reused for each half
            z_ps = ppa.tile([HB, 1], f32, tag="z")

            agc_in = [drpool.tile([HB, D2 + 1], bf, name=f"agc_in{h}") for h in range(2)]
            agc_out = [drpool.tile([N_CORES * HB, D2 + 1], bf, name=f"agc_out{h}")
                       for h in range(2)]

            def flush_half(h):
                """Copy (unnormalized ctx | z) to SBUF, bounce to DRAM, AllGather."""
                aug = wpool.tile([HB, D2 + 1], bf, tag="aug", name=f"aug{h}", bufs=2)
                nc.vector.tensor_copy(aug[:, :D2], ctx_ps[:])
                nc.vector.tensor_copy(aug[:, D2:], z_ps[:])
                nc.sync.dma_start(agc_in[h][:], aug[:])
                nc.gpsimd.collective_compute(
                    "AllGather", mybir.AluOpType.bypass,
                    replica_groups=[list(range(N_CORES))],
                    ins=[agc_in[h][:]], outs=[agc_out[h][:]])

            npf = (len(prefetch) + B_LOC - 2) // (B_LOC - 1)
            for b in range(B_LOC):
                half, bh = b // HB, b % HB
                enc_t = []
                for sc in range(SC):
                    t = epool.tile([128, D2], bf, tag="enc")
                    nc.sync.dma_start(t[:], enc_d[b, sc])
                    enc_t.append(t)
                if b >= 1:   # weight prefetch rides behind each batch row
                    emit_prefetch((b - 1) * npf, b * npf)

                # energy: e[s] = sum_d enc[s, d] * w2[d]   (fp32 accum)
                e_raw = wpool.tile([128, SC], f32, tag="eraw")
                for sc in range(SC):
                    prod = wpool.tile([128, D2], bf, tag="prod")
                    nc.vector.tensor_mul(prod[:], enc_t[sc][:], w2b[:])
                    if sc == SC - 1:
                        nc.vector.tensor_reduce(
                            e_raw[:, sc:sc + 1], prod[:],
                            axis=mybir.AxisListType.X, op=mybir.AluOpType.add)
                    else:
                        dump = dpool.tile([128, D2], bf, tag="dump")
                        nc.scalar.activation(dump[:], prod[:], Copy,
                                             accum_out=e_raw[:, sc:sc + 1])

                # p = exp(e + q_b); weight = max(p, 1) == exp(relu(e + q_b))
                p_exp = wpool.tile([128, SC], f32, tag="pexp")
                nc.scalar.activation(p_exp[:], e_raw[:], Exp, bias=q_bc[:, b:b + 1])
                for sc in range(SC):
                    nc.vector.tensor_scalar_max(
                        A[:, b * SC + sc, bh:bh + 1], p_exp[:, sc:sc + 1], 1.0)

                # context (+ z): PSUM-accumulated masked matmuls, per half
                first = (bh == 0)
                last = (bh == HB - 1)
                for sc in range(SC):
                    for nn in range(D2 // 512):
                        nc.tensor.matmul(
                            ctx_ps[:, nn * 512:(nn + 1) * 512],
                            A[:, b * SC + sc, :],
                            enc_t[sc][:, nn * 512:(nn + 1) * 512],
                            start=(first and sc == 0), stop=(last and sc == SC - 1))
                    nc.tensor.matmul(
                        z_ps[:], A[:, b * SC + sc, :], ones_c[:],
                        start=(first and sc == 0), stop=(last and sc == SC - 1))
                if last:
                    flush_half(half)
            ppa_cm.__exit__(None, None, None)

            # PE warm-keeper over the second-AllGather wait window
            warm_cm = tc.tile_pool(name="psW", bufs=1, space="PSUM")
            warmp = warm_cm.__enter__()
            warm_ps = warmp.tile([1, 128], f32, tag="warm")
            for w in range(60):
                nc.tensor.matmul(warm_ps[:], ones_r[:, :1], ones_r[:],
                                 start=True, stop=True)
            warm_cm.__exit__(None, None, None)

            # ======== post-gather: normalize + transpose ctx into rT ========
            ppb_cm = tc.tile_pool(name="psB", bufs=1, space="PSUM")
            ppb = ppb_cm.__enter__()
            for h in range(2):
                half_sb = wpool.tile([N_CORES * HB, D2 + 1], bf, tag="halfsb",
                                     name=f"half_sb{h}", bufs=2)
                nc.sync.dma_start(half_sb[:], agc_out[h][:])
                inv_z = wpool.tile([N_CORES * HB, 1], f32, tag="invz",
                                   name=f"inv_z{h}", bufs=2)
                nc.vector.reciprocal(inv_z[:], half_sb[:, D2:])
                half_n = wpool.tile([N_CORES * HB, D2], bf, tag="halfn",
                                    name=f"half_n{h}", bufs=2)
                nc.vector.tensor_scalar_mul(half_n[:], half_sb[:, :D2], inv_z[:])
                for rc in range(D2 // 128):
                    tp_ps = ppb.tile([128, N_CORES * HB], bf, tag=f"tp{rc % 2}",
                                     name=f"tp_ps{h}_{rc}")
                    nc.tensor.transpose(tp_ps[:], half_n[:, rc * 128:(rc + 1) * 128],
                                        i64[:N_CORES * HB, :N_CORES * HB])
                    dst = rT[:, rc, :].rearrange("p (k q) -> p k q", k=N_CORES)
                    dst = dst[:, :, h * HB:(h + 1) * HB]
                    src = tp_ps[:].rearrange("p (k q) -> p k q", k=N_CORES)
                    nc.vector.tensor_copy(dst, src)

            # ======== gates: [j, b] = bias + sum_r Wcat[j, r] * rnn_in[r, b] ====
            g_ps = [ppb.tile([128, B], f32, tag=f"g{jc}", name=f"g_ps{jc}")
                    for jc in range(4)]
            for jc in range(4):
                nc.tensor.matmul(g_ps[jc][:], biasg[:, jc * 128:(jc + 1) * 128],
                                 ones_r[:, :B], start=True, stop=False)
                for rc in range(RC):
                    nc.tensor.matmul(g_ps[jc][:],
                                     wt[:, rc, jc * 128:(jc + 1) * 128],
                                     rT[:, rc, :],
                                     start=False, stop=(rc == RC - 1))

            # ======== LSTM cell (j-slice on partitions, b on free) ========
            sig = []
            for idx, jc in enumerate((0, 1, 3)):       # i, f, o gates
                t = opool.tile([128, B], f32, tag=f"sg{idx}", name=f"sg{idx}")
                nc.scalar.activation(t[:], g_ps[jc][:], Tanh, scale=0.5)
                nc.vector.tensor_scalar(t[:], t[:], 0.5, 0.5,
                                        op0=mybir.AluOpType.mult,
                                        op1=mybir.AluOpType.add)
                sig.append(t)
            i_s, f_s, o_s = sig
            g_t = opool.tile([128, B], f32, tag="gt")
            nc.scalar.activation(g_t[:], g_ps[2][:], Tanh)

            fc_ = opool.tile([128, B], f32, tag="fc_")
            nc.vector.tensor_mul(fc_[:], f_s[:], cellT[:])
            ig = opool.tile([128, B], f32, tag="ig")
            nc.vector.tensor_mul(ig[:], i_s[:], g_t[:])
            c_new = opool.tile([128, B], f32, tag="cnew")
            nc.vector.tensor_add(c_new[:], fc_[:], ig[:])
            nc.sync.dma_start(cT_d[:], c_new[:])

            tc_t = opool.tile([128, B], f32, tag="tct")
            nc.scalar.activation(tc_t[:], c_new[:], Tanh)
            h_new = opool.tile([128, B], f32, tag="hnew")
            nc.vector.tensor_mul(h_new[:], o_s[:], tc_t[:])
            nc.sync.dma_start(hT_d[:], h_new[:])
            h_bf = opool.tile([128, B], bf, tag="hbf")
            nc.vector.tensor_copy(h_bf[:], h_new[:])
            ppb_cm.__exit__(None, None, None)

            # ======== AllGather h'^T -> [H, B] ========
            agh_in = drpool.tile([128, B], bf)
            agh_out = drpool.tile([H, B], bf)
            nc.sync.dma_start(agh_in[:], h_bf[:])
            nc.gpsimd.collective_compute(
                "AllGather", mybir.AluOpType.bypass,
                replica_groups=[list(range(N_CORES))],
                ins=[agh_in[:]], outs=[agh_out[:]])

            # PE warm-keeper over the h-AllGather wait window
            warm2_cm = tc.tile_pool(name="psW2", bufs=1, space="PSUM")
            warmp2 = warm2_cm.__enter__()
            warm2_ps = warmp2.tile([1, 128], f32, tag="warm2")
            for w in range(60):
                nc.tensor.matmul(warm2_ps[:], ones_r[:, :1], ones_r[:],
                                 start=True, stop=True)
            warm2_cm.__exit__(None, None, None)

            hT_full = cpool.tile([128, H // 128, B], bf)
            for c in range(H // 128):
                nc.sync.dma_start(hT_full[:, c, :],
                                  agh_out[c * 128:(c + 1) * 128, :])

            # ======== fc: preds[b, v] = sum_h h'[h, b] * fcW[h, v] + fcb ========
            ppfc_cm = tc.tile_pool(name="psC", bufs=2, space="PSUM")
            ppfc = ppfc_cm.__enter__()
            NV = V_LOC // 500
            for nn in range(NV):
                fc_ps = ppfc.tile([B, 500], f32, tag="fcps", name=f"fc_ps{nn}")
                nc.tensor.matmul(fc_ps[:], ones_r[:, :B],
                                 fcb[:, nn * 500:(nn + 1) * 500],
                                 start=True, stop=False)
                for c in range(H // 128):
                    nc.tensor.matmul(fc_ps[:], hT_full[:, c, :],
                                     fcw[:, c, nn * 500:(nn + 1) * 500],
                                     start=False, stop=(c == H // 128 - 1))
                p_sb = opool.tile([B, 500], f32, tag="psb", name=f"p_sb{nn}")
                nc.scalar.copy(p_sb[:], fc_ps[:])
                nc.sync.dma_start(preds_d[:, nn * 500:(nn + 1) * 500], p_sb[:])
            ppfc_cm.__exit__(None, None, None)

    nc.compile()
    return nc


def _prep_inputs(x, encoder_states, hidden, cell, emb_table, energy_W, energy_b,
                 W_ih, W_hh, b_ih, b_hh, fc_W, fc_b):
    """Shard + lay out the full inputs for the 8 cores (host-side, layout only)."""
    enc = np.asarray(encoder_states, np.float32)          # [S, B, D2]
    hid = np.asarray(hidden, np.float32)[0]               # [B, H]
    cel = np.asarray(cell, np.float32)[0]                 # [B, H]
    eW = np.asarray(energy_W, np.float32)[0]              # [3H]
    emb = np.asarray(emb_table, np.float32)[np.asarray(x).astype(np.int64)]  # [B, E]

    w1, w2 = eW[:H], eW[H:]
    w1p = np.ascontiguousarray(w1.reshape(8, 128).T).astype(BF16)        # [128, 8]
    w2b = np.ascontiguousarray(np.broadcast_to(w2.astype(BF16), (128, D2)))
    be = np.full((1, B_LOC), float(np.asarray(energy_b).reshape(-1)[0]), BF16)

    embT = np.ascontiguousarray(emb.T).astype(BF16).reshape(E // 128, 128, B)
    hidT = np.ascontiguousarray(hid.T).astype(BF16).reshape(H // 128, 128, B)

    Wcat = np.concatenate([np.asarray(W_ih, np.float32),
                           np.asarray(W_hh, np.float32)], axis=1)        # [4H, R]
    bias = np.asarray(b_ih, np.float32) + np.asarray(b_hh, np.float32)   # [4H]
    fcW = np.asarray(fc_W, np.float32)                                   # [V, H]
    fcb = np.asarray(fc_b, np.float32)                                   # [V]
    i64 = np.eye(64, dtype=BF16)

    enc_bf = enc.astype(BF16)                                            # [S, B, D2]

    in_maps = []
    for k in range(N_CORES):
        bsl = slice(k * B_LOC, (k + 1) * B_LOC)
        enc_k = np.ascontiguousarray(
            enc_bf[:, bsl, :].transpose(1, 0, 2)).reshape(B_LOC, SC, 128, D2)
        jidx = np.concatenate([np.arange(g * H + k * GJ, g * H + (k + 1) * GJ)
                               for g in range(4)])                        # [512]
        wt_k = np.ascontiguousarray(Wcat[jidx].T).astype(BF16).reshape(RC, 128, 512)
        biasg_k = bias[jidx].astype(BF16).reshape(1, 512)
        cellT_k = np.ascontiguousarray(cel[:, k * GJ:(k + 1) * GJ].T).astype(np.float32)
        vsl = slice(k * V_LOC, (k + 1) * V_LOC)
        fcw_k = np.ascontiguousarray(fcW[vsl].T).astype(BF16).reshape(H // 128, 128, V_LOC)
        fcb_k = fcb[vsl].astype(BF16).reshape(1, V_LOC)
        hq_k = np.ascontiguousarray(hidT[:, :, bsl])
        in_maps.append({
            "enc": enc_k, "w2b": w2b, "w1p": w1p, "be": be, "hq": hq_k,
            "embT": embT, "hidT": hidT, "cellT": cellT_k,
            "wt": wt_k, "biasg": biasg_k,
            "fcw": fcw_k, "fcb": fcb_k, "i64": i64,
        })
    return in_maps


def kernel(**inputs):
    if "nc" not in _CACHE:
        _CACHE["nc"] = _build()
    nc = _CACHE["nc"]
    in_maps = _prep_inputs(**inputs)
    res = run_bass_kernel_spmd(nc, in_maps, core_ids=list(range(N_CORES)),
                               trace=False)
    preds = np.concatenate([res.results[k]["preds"] for k in range(N_CORES)],
                           axis=1)                                       # [B, V]
    h_new = np.concatenate([res.results[k]["hT"].T for k in range(N_CORES)],
                           axis=1)[None]                                 # [1, B, H]
    c_new = np.concatenate([res.results[k]["cT"].T for k in range(N_CORES)],
                           axis=1)[None]
    return preds, h_new, c_new
